# revision 67
# baseline (speedup 1.0000x reference)
"""GPS layer (GCN + dense Performer attention + FFN) on 8 Trainium2 cores.

v4 strategy (per core, rows R=1024 of N=8192 nodes):
  - GCN segment-sum as dense matmul, reassociated: hl = (A @ h) @ W_gcn.
    A (x4) ships fp8-e4m3; h (x0.25) ships as fp8 value + fp8 residual, and
    the A@h contraction runs two DoubleRow chains (value + residual) so the
    fp8 quantization error of h cancels to ~0.1%.
  - Performer features qf^T/kf^T produced directly from h1^T with
    host-folded projections (Wq@RF^T, Wk@RF^T) in bf16; score matmuls bf16.
  - softmax without row-max: exp(raw/16 - 4.5) fits fp8-e5m2; the slab is
    e5m2 (1 byte -> the full [64 chunk, 1024 row] slab fits SBUF, letting
    exp start as soon as the first half of kf arrives).
  - V is host-folded through Wo (+bo): v' = h1 @ (Wv@Wo) + (bv@Wo + bo);
    softmax rows sum to 1 so +bo rides inside v'.  v' ships e4m3 PLUS its
    e4m3 residual; the P@V numerator is one 64-instruction DoubleRow
    accumulation (value part then residual part) into a single PSUM bank.
  - kf^T (bf16) and v' (2x e4m3) all-gather in two halves, fired as soon
    as their half of h1 exists.
  - ACT keeps one table per epoch: LN1 uses Sqrt (all before the first
    exp); LN2/LN3 compute rsqrt on DVE (Newton + bit-trick seed) so the
    exp table never reloads.
  - Input streaming is spread over three DMA rings: hm + at[4..7] on
    scalar, hr + weights on sync, at[0..3] on gpsimd (before any
    collective trigger blocks that ring).
  - The attention/FFN tail is software-pipelined: numerator chain rb+1
    issues between the drain of rb and the FFN tail of rb, keeping PE busy
    during the DVE layer-norm work.
"""

import os
import sys

sys.path.insert(0, "/opt/trn_rl_repo")
os.environ.setdefault("MYCRO_LOCAL_CACHE", "1")

import numpy as np
import ml_dtypes

import concourse.bass as bass
import concourse.tile as tile
from concourse import bacc, mybir
from concourse.bass_utils import run_bass_kernel_spmd
from concourse.masks import make_identity

f32 = mybir.dt.float32
bf16 = mybir.dt.bfloat16
f8e4 = mybir.dt.float8e4
f8e5 = mybir.dt.float8e5
DR = mybir.MatmulPerfMode.DoubleRow
BF = ml_dtypes.bfloat16
F8 = ml_dtypes.float8_e4m3

N, D, F, M = 8192, 256, 512, 256
NCORES = 8
R = N // NCORES          # rows per core (1024)
RB = R // 128            # row blocks per core (8)
KC = D // 128            # feature chunks (2)
NCH = N // 128           # node chunks (64)
FC = F // 128            # ffn chunks (4)
VA = 260                 # v free dim: 256 features + ones col + pad
VA2 = 2 * VA             # v' + its e4m3 residual, packed side by side
EPS = 1e-5
RH = R // 2              # rows per collective half (512)
SHIFT = -4.5             # exp shift: exp(raw/16 - 4.5) in [6e-7, 4.1e4]


def _build():
    nc = bacc.Bacc("TRN2", target_bir_lowering=False, debug=False,
                   num_devices=NCORES)

    def inp(name, shape, dt):
        return nc.dram_tensor(name, shape, dt, kind="ExternalInput")

    at_h = inp("at", [RB, 128, NCH, 128], f8e4)   # at[rb,p,k,f] = 4*A[r0+rb*128+f, k*128+p]
    hm_h = inp("hm", [128, NCH, D], f8e4)         # h * 0.25, pre-transposed p,k,d
    hres_h = inp("hres", [R, D], f32)             # h rows + b_gcn
    wgcn_h = inp("wgcn", [D, D], bf16)
    rfwq_h = inp("rfwq", [D, M], bf16)            # Wq @ RF^T
    rfwk_h = inp("rfwk", [D, M], bf16)            # Wk @ RF^T
    wvwo_h = inp("wvwo", [D, D], bf16)            # Wv @ Wo
    w1_h = inp("w1", [D, F], bf16)
    w2_h = inp("w2", [F, D], bf16)
    rfbq_h = inp("rfbq", [M, 1], f32)             # RF @ bq
    rfbk_h = inp("rfbk", [M, 1], f32)             # RF @ bk
    bvor_h = inp("bvor", [1, D], bf16)            # bv @ Wo + bo
    b1r_h = inp("b1r", [1, F], bf16)
    b2r_h = inp("b2r", [1, D], bf16)
    gb_h = {}
    for nm in ("g1", "be1", "g2", "be2", "g3", "be3"):
        gb_h[nm] = inp(nm, [1, D], bf16)

    out_h = nc.dram_tensor("out", [R, D], f32, kind="ExternalOutput")

    with tile.TileContext(nc) as tc:
        _body(tc, at_h, hm_h, hres_h, wgcn_h, rfwq_h, rfwk_h, wvwo_h,
              w1_h, w2_h, rfbq_h, rfbk_h, bvor_h, b1r_h, b2r_h, gb_h, out_h)
    nc.compile()
    return nc


def _nr_rsqrt(nc, pool, v_ap, out_ap, magic_i, W=1):
    """out = 1/sqrt(v) for [128,W] f32 via bit-trick seed + 2 Newton steps.
    All on DVE (no ACT table use). W>1 batches independent values."""
    y = pool.tile([128, 4 * W], f32, tag=f"nr{W}", name=f"nr{W}")
    yi = y.bitcast(mybir.dt.int32)
    vi = v_ap.bitcast(mybir.dt.int32)
    y0, hv, t = yi[:, 0:W], y[:, W:2 * W], y[:, 2 * W:3 * W]
    nc.vector.tensor_scalar(out=y0, in0=vi, scalar1=1, scalar2=None,
                            op0=mybir.AluOpType.arith_shift_right)
    nc.vector.scalar_tensor_tensor(out=y0, in0=magic_i, scalar=0,
                                   in1=y0,
                                   op0=mybir.AluOpType.bypass,
                                   op1=mybir.AluOpType.subtract)
    nc.vector.tensor_scalar(out=hv, in0=v_ap, scalar1=0.5, scalar2=None,
                            op0=mybir.AluOpType.mult)
    y0f = y[:, 0:W]
    for _ in range(2):
        nc.vector.tensor_mul(t, y0f, y0f)
        nc.vector.tensor_mul(t, t, hv)
        nc.vector.tensor_scalar(out=t, in0=t,
                                scalar1=-1.0, scalar2=1.5,
                                op0=mybir.AluOpType.mult,
                                op1=mybir.AluOpType.add)
        nc.vector.tensor_mul(y0f, y0f, t)
    nc.vector.tensor_copy(out_ap, y0f)


def _body(tc, at_h, hm_h, hres_h, wgcn_h, rfwq_h, rfwk_h, wvwo_h,
          w1_h, w2_h, rfbq_h, rfbk_h, bvor_h, b1r_h, b2r_h, gb_h, out_h):
    from contextlib import ExitStack
    nc = tc.nc
    Exp = mybir.ActivationFunctionType.Exp
    Sqrt = mybir.ActivationFunctionType.Sqrt
    Copy = mybir.ActivationFunctionType.Copy

    with ExitStack() as octx:
        const = octx.enter_context(tc.tile_pool(name="const", bufs=1))
        persist = octx.enter_context(tc.tile_pool(name="persist", bufs=1))
        dram = octx.enter_context(tc.tile_pool(name="dram", bufs=1, space="DRAM"))
        sc = octx.enter_context(tc.tile_pool(name="sc", bufs=2))
        big_ps = octx.enter_context(tc.tile_pool(name="big_ps", bufs=2, space="PSUM"))
        acc_ps = octx.enter_context(tc.tile_pool(name="acc_ps", bufs=1, space="PSUM"))
        tp_ps = octx.enter_context(tc.tile_pool(name="tp_ps", bufs=1, space="PSUM"))
        num_ps = octx.enter_context(tc.tile_pool(name="num_ps", bufs=1, space="PSUM"))

        # ---- const tiles (allocation only; DMA issue order is controlled
        #      below so the sync ring streams hr before the fat weights) ----
        def wtile(chunks, width, name, dt=bf16):
            return const.tile([128, chunks, width], dt, tag=name, name=name)

        wgcn_sb = wtile(KC, D, "wgcn")
        rfwq_sb = wtile(KC, M, "rfwq")
        rfwk_sb = wtile(KC, M, "rfwk")
        wvwo_sb = wtile(KC, D, "wvwo")
        w1_sb = wtile(KC, F, "w1")
        w2_sb = wtile(FC, D, "w2")

        def load_w(t, h):
            nc.sync.dma_start(t[:], h[:, :].rearrange("(c p) w -> p c w", p=128))

        rfbq_sb = const.tile([128, KC], f32, tag="rfbq")
        rfbk_sb = const.tile([128, KC], f32, tag="rfbk")
        bvor_sb = const.tile([1, D], bf16, tag="bvor")
        b1r_sb = const.tile([1, F], bf16, tag="b1r")
        b2r_sb = const.tile([1, D], bf16, tag="b2r")
        gb_sb = {nm: const.tile([128, D], bf16, tag=nm, name=nm)
                 for nm in gb_h}

        ones_k1 = const.tile([1, 128], bf16, tag="ones")
        nc.vector.memset(ones_k1[:], 1.0)
        ident_bf = const.tile([128, 128], bf16, tag="ident")
        make_identity(nc, ident_bf[:])
        eps_t = const.tile([128, 1], f32, tag="eps")
        nc.vector.memset(eps_t[:], EPS)
        shift_t = const.tile([128, 1], f32, tag="shift")
        nc.vector.memset(shift_t[:], SHIFT)
        magic_t = const.tile([128, 2], mybir.dt.int32, tag="magic")
        nc.vector.memset(magic_t[:], 0x5F3759DF)

        # ---- persistent activations ----
        # kf/qf ship as fp8 value + fp8 residual so the score matmuls run
        # three DoubleRow chains (v@v, v@r, r@v) — faster AND more accurate
        # than a single bf16 pair.
        k8_sb = persist.tile([128, KC, N], f8e4, tag="k8")
        kr_sb = persist.tile([128, KC, N], f8e4, tag="kr")
        h1_sb = persist.tile([128, RB, D], f32, tag="h1")
        h1t_sb = persist.tile([128, KC, R], bf16, tag="h1t")
        q8_sb = persist.tile([128, KC, R], f8e4, tag="q8")
        qr_sb = persist.tile([128, KC, R], f8e4, tag="qr")
        vaug_sb = persist.tile([128, NCH, VA2], f8e4, tag="vaug")
        hres_sb = persist.tile([128, RB, D], f32, tag="hres")

        # ---- collective DRAM buffers (two halves) ----
        # kft carries [k8 | kr] stacked: [2, M, RH] fp8 per half
        kft_loc = [dram.tile([2 * M, RH], f8e4, tag=f"kft_loc{h}", name=f"kft_loc{h}")
                   for h in range(2)]
        vaug_loc = [dram.tile([RH, VA2], f8e4, tag=f"vaug_loc{h}", name=f"vaug_loc{h}")
                    for h in range(2)]
        kft_all = [dram.tile([NCORES * 2 * M, RH], f8e4, tag=f"kft_all{h}",
                              name=f"kft_all{h}", addr_space="Shared")
                   for h in range(2)]
        vaug_all = [dram.tile([NCORES * RH, VA2], f8e4, tag=f"vaug_all{h}",
                               name=f"vaug_all{h}", addr_space="Shared")
                    for h in range(2)]

        def fire_kft(half):
            nc.gpsimd.collective_compute(
                "AllGather", mybir.AluOpType.bypass,
                replica_groups=[list(range(NCORES))],
                ins=[kft_loc[half][:].opt()], outs=[kft_all[half][:].opt()])

        def fire_vaug(half):
            nc.gpsimd.collective_compute(
                "AllGather", mybir.AluOpType.bypass,
                replica_groups=[list(range(NCORES))],
                ins=[vaug_loc[half][:].opt()], outs=[vaug_all[half][:].opt()])

        def load_kfa(half):
            r0 = half * RH
            for piece, dst in ((0, k8_sb), (1, kr_sb)):
                for mc in range(KC):
                    nc.sync.dma_start(
                        dst[:, mc, :].rearrange(
                            "p (c rr) -> p c rr", c=NCORES)[:, :, r0:r0 + RH],
                        kft_all[half][:, :].rearrange(
                            "(c t m p) r -> t m p c r",
                            t=2, m=KC, p=128)[piece][mc])

        def load_vaug(half):
            for c in range(NCORES):
                cg0 = c * 8 + half * 4
                nc.sync.dma_start(
                    vaug_sb[:, cg0:cg0 + 4, :],
                    vaug_all[half][c * 4 * 128:(c * 4 + 4) * 128, :].rearrange(
                        "(l p) v -> p l v", p=128))

        # ---- LN helpers ----
        def _ln_tail(x_sb, out_sb, mv, g_bc, be_bc):
            t = sc.tile([128, D], f32, tag="ln_t")
            nc.vector.scalar_tensor_tensor(
                out=t[:], in0=x_sb, scalar=mv[:, 0:1], in1=g_bc,
                op0=mybir.AluOpType.subtract, op1=mybir.AluOpType.mult)
            nc.vector.scalar_tensor_tensor(
                out=out_sb, in0=t[:], scalar=mv[:, 1:2], in1=be_bc,
                op0=mybir.AluOpType.mult, op1=mybir.AluOpType.add)

        def ln_act(x_sb, out_sb, g_bc, be_bc):
            stats = sc.tile([128, 6], f32, tag="ln_stats")
            nc.vector.bn_stats(stats[:], x_sb)
            mv = sc.tile([128, 2], f32, tag="ln_mv", bufs=4)
            nc.vector.bn_aggr(mv[:], stats[:])
            nc.scalar.activation(mv[:, 1:2], mv[:, 1:2], Sqrt, bias=eps_t[:])
            nc.vector.reciprocal(mv[:, 1:2], mv[:, 1:2])
            _ln_tail(x_sb, out_sb, mv, g_bc, be_bc)

        def ln_dve(x_sb, out_sb, g_bc, be_bc):
            stats = sc.tile([128, 6], f32, tag="ln_stats")
            nc.vector.bn_stats(stats[:], x_sb)
            mv = sc.tile([128, 2], f32, tag="ln_mv", bufs=4)
            nc.vector.bn_aggr(mv[:], stats[:])
            nc.vector.tensor_scalar(out=mv[:, 1:2], in0=mv[:, 1:2],
                                    scalar1=eps_t[:, 0:1], scalar2=None,
                                    op0=mybir.AluOpType.add)
            _nr_rsqrt(nc, sc, mv[:, 1:2], mv[:, 1:2], magic_t[:, 0:1])
            _ln_tail(x_sb, out_sb, mv, g_bc, be_bc)

        def ln_pair(xa, xb, oa, ob, g_bc, be_bc, use_act):
            """Two independent LayerNorms with interleaved stages."""
            sta = sc.tile([128, 6], f32, tag="ln_stats")
            stb = sc.tile([128, 6], f32, tag="ln_stats")
            nc.vector.bn_stats(sta[:], xa)
            nc.vector.bn_stats(stb[:], xb)
            mva = sc.tile([128, 2], f32, tag="ln_mv", bufs=4, name="mva")
            mvb = sc.tile([128, 2], f32, tag="ln_mv", bufs=4, name="mvb")
            nc.vector.bn_aggr(mva[:], sta[:])
            nc.vector.bn_aggr(mvb[:], stb[:])
            if use_act:
                nc.scalar.activation(mva[:, 1:2], mva[:, 1:2], Sqrt,
                                     bias=eps_t[:])
                nc.scalar.activation(mvb[:, 1:2], mvb[:, 1:2], Sqrt,
                                     bias=eps_t[:])
                nc.vector.reciprocal(mva[:, 1:2], mva[:, 1:2])
                nc.vector.reciprocal(mvb[:, 1:2], mvb[:, 1:2])
            else:
                vv = sc.tile([128, 2], f32, tag="vv")
                nc.vector.tensor_copy(vv[:, 0:1], mva[:, 1:2])
                nc.vector.tensor_copy(vv[:, 1:2], mvb[:, 1:2])
                nc.vector.tensor_scalar(out=vv[:], in0=vv[:],
                                        scalar1=eps_t[:, 0:1], scalar2=None,
                                        op0=mybir.AluOpType.add)
                _nr_rsqrt(nc, sc, vv[:], vv[:], magic_t[:, 0:2], W=2)
                nc.vector.tensor_copy(mva[:, 1:2], vv[:, 0:1])
                nc.vector.tensor_copy(mvb[:, 1:2], vv[:, 1:2])
            ta = sc.tile([128, D], f32, tag="ln_t")
            tb = sc.tile([128, D], f32, tag="ln_t")
            nc.vector.scalar_tensor_tensor(
                out=ta[:], in0=xa, scalar=mva[:, 0:1], in1=g_bc,
                op0=mybir.AluOpType.subtract, op1=mybir.AluOpType.mult)
            nc.vector.scalar_tensor_tensor(
                out=tb[:], in0=xb, scalar=mvb[:, 0:1], in1=g_bc,
                op0=mybir.AluOpType.subtract, op1=mybir.AluOpType.mult)
            nc.vector.scalar_tensor_tensor(
                out=oa, in0=ta[:], scalar=mva[:, 1:2], in1=be_bc,
                op0=mybir.AluOpType.mult, op1=mybir.AluOpType.add)
            nc.vector.scalar_tensor_tensor(
                out=ob, in0=tb[:], scalar=mvb[:, 1:2], in1=be_bc,
                op0=mybir.AluOpType.mult, op1=mybir.AluOpType.add)

        # ---- qkv segments ----
        def kft_half(half):
            r0 = half * RH
            for mc in range(KC):
                mm = acc_ps.tile([128, F], f32, tag="acc")
                for j in range(KC):
                    nc.tensor.matmul(mm[:, 0:RH],
                                     rfwk_sb[:, j, mc * 128:(mc + 1) * 128],
                                     h1t_sb[:, j, r0:r0 + RH],
                                     start=(j == 0), stop=(j == KC - 1))
                k8sl = sc.tile([128, RH], f8e4, tag="k8sl")
                nc.vector.tensor_scalar(
                    out=k8sl[:], in0=mm[:, 0:RH],
                    scalar1=rfbk_sb[:, mc:mc + 1], scalar2=None,
                    op0=mybir.AluOpType.add)
                krsl = sc.tile([128, RH], f8e4, tag="krsl")
                nc.vector.scalar_tensor_tensor(
                    out=krsl[:], in0=mm[:, 0:RH],
                    scalar=rfbk_sb[:, mc:mc + 1], in1=k8sl[:],
                    op0=mybir.AluOpType.add, op1=mybir.AluOpType.subtract)
                nc.sync.dma_start(
                    kft_loc[half][mc * 128:(mc + 1) * 128, :], k8sl[:])
                nc.sync.dma_start(
                    kft_loc[half][M + mc * 128:M + (mc + 1) * 128, :], krsl[:])
            fire_kft(half)

        def qft_half(half):
            r0 = half * RH
            for mc in range(KC):
                mm = acc_ps.tile([128, F], f32, tag="acc")
                for j in range(KC):
                    nc.tensor.matmul(mm[:, 0:RH],
                                     rfwq_sb[:, j, mc * 128:(mc + 1) * 128],
                                     h1t_sb[:, j, r0:r0 + RH],
                                     start=(j == 0), stop=(j == KC - 1))
                nc.vector.tensor_scalar(
                    out=q8_sb[:, mc, r0:r0 + RH], in0=mm[:, 0:RH],
                    scalar1=rfbq_sb[:, mc:mc + 1], scalar2=None,
                    op0=mybir.AluOpType.add)
                nc.vector.scalar_tensor_tensor(
                    out=qr_sb[:, mc, r0:r0 + RH], in0=mm[:, 0:RH],
                    scalar=rfbq_sb[:, mc:mc + 1], in1=q8_sb[:, mc, r0:r0 + RH],
                    op0=mybir.AluOpType.add, op1=mybir.AluOpType.subtract)

        def v_half(half):
            for b in range(half * (RB // 2), (half + 1) * (RB // 2)):
                vps = acc_ps.tile([128, F], f32, tag="acc")
                for j in range(KC):
                    nc.tensor.matmul(vps[:, 0:D],
                                     h1t_sb[:, j, b * 128:(b + 1) * 128],
                                     wvwo_sb[:, j, :],
                                     start=(j == 0), stop=False)
                nc.tensor.matmul(vps[:, 0:D], ones_k1[:], bvor_sb[:],
                                 start=False, stop=True)
                # v' in e4m3 plus its quantization residual, packed [v' | r];
                # the ones column lives in v' only.
                vrow = sc.tile([128, VA2], f8e4, tag="vrow")
                nc.vector.tensor_copy(vrow[:, 0:D], vps[:, 0:D])
                nc.vector.memset(vrow[:, D:VA], 0.0)
                nc.vector.memset(vrow[:, D:D + 1], 1.0)
                nc.vector.tensor_sub(vrow[:, VA:VA + D], vps[:, 0:D],
                                     vrow[:, 0:D])
                nc.vector.memset(vrow[:, VA + D:VA2], 0.0)
                lb = b - half * (RB // 2)
                # scalar ring: the sync ring is busy with kfa loads here
                nc.scalar.dma_start(
                    vaug_loc[half][lb * 128:(lb + 1) * 128, :], vrow[:])
            fire_vaug(half)

        # ---- scores: one (rc, half) quarter = 32 chunks, 3x fp8 DR ----
        def scores_quarter(slab_sb, rc, half):
            r0 = rc * RH
            cgs = [c * 8 + half * 4 + l for c in range(NCORES) for l in range(4)]
            for i in range(0, len(cgs), 2):
                ps = big_ps.tile([128, 2, RH], f32, tag="big")
                for t in range(2):
                    cg = cgs[i + t]
                    csl = slice(cg * 128, (cg + 1) * 128)
                    rsl = slice(r0, r0 + RH)
                    nc.tensor.matmul(ps[:, t, :], k8_sb[:, :, csl],
                                     q8_sb[:, :, rsl],
                                     start=True, stop=False, perf_mode=DR)
                    nc.tensor.matmul(ps[:, t, :], k8_sb[:, :, csl],
                                     qr_sb[:, :, rsl],
                                     start=False, stop=False, perf_mode=DR)
                    nc.tensor.matmul(ps[:, t, :], kr_sb[:, :, csl],
                                     q8_sb[:, :, rsl],
                                     start=False, stop=True, perf_mode=DR)
                cg0 = cgs[i]
                nc.scalar.activation(
                    slab_sb[:, cg0:cg0 + 2, r0:r0 + RH], ps[:], Exp,
                    bias=shift_t[:], scale=1.0 / 16.0)

        # ---- attention tail, software-pipelined over row blocks ----
        def num_chain(slab_sb, rb):
            # two interleaved DR accumulation groups (value -> bank 0,
            # residual -> bank 1); consecutive pairs share the slab lhsT
            nps = num_ps.tile([128, 2, RH], f32, tag="num", name=f"nps{rb}")
            for cg in range(0, NCH, 2):
                lhs = slab_sb[:, cg:cg + 2, rb * 128:(rb + 1) * 128]
                nc.tensor.matmul(
                    nps[:, 0, 0:VA], lhs, vaug_sb[:, cg:cg + 2, 0:VA],
                    start=(cg == 0), stop=(cg == NCH - 2), perf_mode=DR)
                nc.tensor.matmul(
                    nps[:, 1, 0:VA], lhs, vaug_sb[:, cg:cg + 2, VA:VA2],
                    start=(cg == 0), stop=(cg == NCH - 2), perf_mode=DR)
            return nps

        def drain(nps, rb):
            rec = sc.tile([128, 1], f32, tag="rec")
            nc.vector.reciprocal(rec[:], nps[:, 0, D:D + 1])
            t = sc.tile([128, D], f32, tag="numsum")
            nc.vector.scalar_tensor_tensor(
                out=t[:], in0=nps[:, 0, 0:D], scalar=rec[:, 0:1],
                in1=h1_sb[:, rb, :],
                op0=mybir.AluOpType.mult, op1=mybir.AluOpType.add)
            x2 = sc.tile([128, D], f32, tag="x2", name=f"x2_{rb}", bufs=4)
            nc.vector.scalar_tensor_tensor(
                out=x2[:], in0=nps[:, 1, 0:D], scalar=rec[:, 0:1],
                in1=t[:],
                op0=mybir.AluOpType.mult, op1=mybir.AluOpType.add)
            return x2

        def ffn_tail2(x2a, x2b, rba, rbb, use_act):
            # two row blocks with interleaved stages: while one side's
            # serial LN/copy chain waits, the other's ops fill the queues
            Relu = mybir.ActivationFunctionType.Relu

            def cp(dst, src):
                if use_act:
                    nc.scalar.activation(dst, src, Copy)
                else:
                    nc.vector.tensor_copy(dst, src)

            h2a = sc.tile([128, D], f32, tag="h2", name="h2a")
            h2b = sc.tile([128, D], f32, tag="h2", name="h2b")
            ln_pair(x2a[:], x2b[:], h2a[:], h2b[:],
                    gb_sb["g2"][:], gb_sb["be2"][:], use_act)
            h2bfa = sc.tile([128, D], bf16, tag="h2bf", name="h2bfa")
            h2bfb = sc.tile([128, D], bf16, tag="h2bf", name="h2bfb")
            cp(h2bfa[:], h2a[:])
            cp(h2bfb[:], h2b[:])
            h2Ta = sc.tile([128, KC, 128], bf16, tag="h2T", name="h2Ta")
            h2Tb = sc.tile([128, KC, 128], bf16, tag="h2T", name="h2Tb")
            for src, dst in ((h2bfa, h2Ta), (h2bfb, h2Tb)):
                for j in range(KC):
                    tp = tp_ps.tile([128, 128], bf16, tag="tp")
                    nc.tensor.transpose(tp[:], src[:, j * 128:(j + 1) * 128],
                                        ident_bf[:])
                    cp(dst[:, j, :], tp[:])

            usa = sc.tile([128, F], bf16, tag="usb", name="usa")
            usb = sc.tile([128, F], bf16, tag="usb", name="usb")
            for h2T, u_sb in ((h2Ta, usa), (h2Tb, usb)):
                ups = acc_ps.tile([128, F], f32, tag="acc")
                for j in range(KC):
                    nc.tensor.matmul(ups[:], h2T[:, j, :], w1_sb[:, j, :],
                                     start=(j == 0), stop=False)
                nc.tensor.matmul(ups[:], ones_k1[:], b1r_sb[:],
                                 start=False, stop=True)
                if use_act:
                    nc.scalar.activation(u_sb[:], ups[:], Relu)
                else:
                    nc.vector.tensor_scalar_max(u_sb[:], ups[:], 0.0)

            uTa = sc.tile([128, FC, 128], bf16, tag="uT", name="uTa")
            uTb = sc.tile([128, FC, 128], bf16, tag="uT", name="uTb")
            for u_sb, uT in ((usa, uTa), (usb, uTb)):
                for jf in range(FC):
                    tp = tp_ps.tile([128, 128], bf16, tag="tp")
                    nc.tensor.transpose(
                        tp[:], u_sb[:, jf * 128:(jf + 1) * 128], ident_bf[:])
                    cp(uT[:, jf, :], tp[:])

            x3a = sc.tile([128, D], f32, tag="x3", name="x3a")
            x3b = sc.tile([128, D], f32, tag="x3", name="x3b")
            for uT, h2, x3 in ((uTa, h2a, x3a), (uTb, h2b, x3b)):
                o2 = acc_ps.tile([128, F], f32, tag="acc")
                for jf in range(FC):
                    nc.tensor.matmul(o2[:, 0:D], uT[:, jf, :], w2_sb[:, jf, :],
                                     start=(jf == 0), stop=False)
                nc.tensor.matmul(o2[:, 0:D], ones_k1[:], b2r_sb[:],
                                 start=False, stop=True)
                nc.vector.tensor_add(x3[:], o2[:, 0:D], h2[:])
            oa = sc.tile([128, D], f32, tag="osb", name="oa")
            ob = sc.tile([128, D], f32, tag="osb", name="ob")
            ln_pair(x3a[:], x3b[:], oa[:], ob[:],
                    gb_sb["g3"][:], gb_sb["be3"][:], use_act)
            nc.sync.dma_start(out_h[rba * 128:(rba + 1) * 128, :], oa[:])
            nc.sync.dma_start(out_h[rbb * 128:(rbb + 1) * 128, :], ob[:])

        # ================= main schedule =================
        # Phase-1 pool: hm/hr/at die after the GCN; the slab pool reuses
        # their SBUF afterwards (LIFO stack allocation).
        with ExitStack() as p1ctx:
            p1 = p1ctx.enter_context(tc.tile_pool(name="p1", bufs=1))
            hm_sb = p1.tile([128, NCH, D], f8e4, tag="hm")

            # bufs=7: every at trigger except at7 has a free buffer at issue
            # time, so no ring ever blocks on a buffer-wait (idle PE drops
            # its p-state and runs the next chain 1.5x slower).
            at_ts = [p1.tile([128, NCH, 128], f8e4, tag="at", bufs=7,
                             name=f"at{rb}")
                     for rb in range(RB)]
            for rb in (0, 2, 4, 6):
                nc.gpsimd.dma_start(at_ts[rb][:], at_h[rb])
            nc.scalar.dma_start(at_ts[5][:], at_h[5])
            # sync ring: hm first (rb0 gate), tiny consts, then at-odd
            # interleaved with progressively-later-needed weights
            for q in range(4):
                nc.sync.dma_start(hm_sb[:, q * 16:(q + 1) * 16, :],
                                  hm_h[:, q * 16:(q + 1) * 16, :])
            for j in range(KC):
                nc.sync.dma_start(rfbq_sb[:, j:j + 1],
                                  rfbq_h[j * 128:(j + 1) * 128, :])
                nc.sync.dma_start(rfbk_sb[:, j:j + 1],
                                  rfbk_h[j * 128:(j + 1) * 128, :])
            nc.sync.dma_start(bvor_sb[:], bvor_h[:, :])
            nc.sync.dma_start(b1r_sb[:], b1r_h[:, :])
            nc.sync.dma_start(b2r_sb[:], b2r_h[:, :])
            for nm, h in gb_h.items():
                bcast = bass.AP(tensor=h.ap().tensor, offset=h.ap().offset,
                                ap=[[0, 128]] + list(h.ap().ap[1:]))
                nc.sync.dma_start(gb_sb[nm][:], bcast)
            load_w(wgcn_sb, wgcn_h)
            nc.sync.dma_start(at_ts[1][:], at_h[1])
            nc.sync.dma_start(
                hres_sb[:], hres_h[:, :].rearrange("(rb p) d -> p rb d", p=128))
            nc.sync.dma_start(at_ts[3][:], at_h[3])
            load_w(rfwq_sb, rfwq_h)
            load_w(rfwk_sb, rfwk_h)
            nc.sync.dma_start(at_ts[7][:], at_h[7])
            load_w(wvwo_sb, wvwo_h)
            load_w(w1_sb, w1_h)
            load_w(w2_sb, w2_h)

            def gcn_tail(rb, hl_ps):
                hl = hl_ps[:, 0, 0:D]
                hl_bf = sc.tile([128, D], bf16, tag="hlbf")
                nc.vector.tensor_copy(hl_bf[:], hl)
                hlT = sc.tile([128, KC, 128], bf16, tag="hlT")
                for j in range(KC):
                    tp = tp_ps.tile([128, 128], bf16, tag="tp")
                    nc.tensor.transpose(tp[:], hl_bf[:, j * 128:(j + 1) * 128],
                                        ident_bf[:])
                    nc.vector.tensor_copy(hlT[:, j, :], tp[:])
                # W_gcn result lands in the unused second bank of hl_ps, so
                # consecutive GCN tails don't serialize on the acc pool
                hloc = hl_ps[:, 1, 0:D]
                for j in range(KC):
                    nc.tensor.matmul(hloc, hlT[:, j, :],
                                     wgcn_sb[:, j, :],
                                     start=(j == 0), stop=(j == KC - 1))
                x1 = sc.tile([128, D], f32, tag="x1")
                nc.vector.tensor_add(x1[:], hloc, hres_sb[:, rb, :])
                ln_act(x1[:], h1_sb[:, rb, :], gb_sb["g1"][:], gb_sb["be1"][:])
                h1bf = sc.tile([128, D], bf16, tag="h1bf")
                nc.vector.tensor_copy(h1bf[:], h1_sb[:, rb, :])
                for j in range(KC):
                    tp = tp_ps.tile([128, 128], bf16, tag="tp")
                    nc.tensor.transpose(tp[:], h1bf[:, j * 128:(j + 1) * 128],
                                        ident_bf[:])
                    nc.scalar.activation(
                        h1t_sb[:, j, rb * 128:(rb + 1) * 128], tp[:], Copy)

            # software-pipelined GCN: chain rb issues before the tail of
            # rb-1 so the DVE/ACT tail hides under the next PE chain
            prev_hl = None
            for rb in range(RB):
                hl_ps = big_ps.tile([128, 2, RH], f32, tag="big")
                hl = hl_ps[:, 0, 0:D]
                for k in range(0, NCH, 2):
                    nc.tensor.matmul(hl, at_ts[rb][:, k:k + 2, :],
                                     hm_sb[:, k:k + 2, :],
                                     start=(k == 0), stop=(k == NCH - 2),
                                     perf_mode=DR)
                if prev_hl is not None:
                    gcn_tail(rb - 1, prev_hl)
                prev_hl = hl_ps
                if rb == RB // 2:
                    kft_half(0)
                    qft_half(0)
            gcn_tail(RB - 1, prev_hl)
            # half-B projections inside p1 scope so the pool teardown
            # overlaps their PE work
            kft_half(1)
            qft_half(1)

        with ExitStack() as p2ctx:
            slabp = p2ctx.enter_context(tc.tile_pool(name="slabp", bufs=1))
            slab_sb = slabp.tile([128, NCH, R], f8e5, tag="slab")
            # kfa loads go on the sync ring only after BOTH kft stores have
            # been issued, so the B stores are not stuck behind the A loads
            load_kfa(0)
            load_kfa(1)
            # v' blocks first: they fill the PE window while the kf
            # collectives are still in flight, and get the vaug gathers
            # moving early
            v_half(0)
            v_half(1)
            scores_quarter(slab_sb, 0, 0)
            scores_quarter(slab_sb, 1, 0)
            load_vaug(0)
            load_vaug(1)
            scores_quarter(slab_sb, 0, 1)   # needs kfa-B
            scores_quarter(slab_sb, 1, 1)

            # software-pipelined paired tail: two chains + drains, then the
            # previous pair's FFN (interleaved a/b) under the next chains.
            # ACT offload only for late pairs (after the last exp retired).
            pend = None
            for p in range(RB // 2):
                a, b = 2 * p, 2 * p + 1
                npsA = num_chain(slab_sb, a)
                x2a = drain(npsA, a)
                npsB = num_chain(slab_sb, b)
                x2b = drain(npsB, b)
                if pend is not None:
                    ffn_tail2(*pend)
                # all pair tails execute after the last exp retires (nums
                # gate on exp-rc0B; exps end before the first tail runs),
                # so ACT offload is safe everywhere
                pend = (x2a, x2b, a, b, True)
            ffn_tail2(*pend)


_NC_CACHE = None


def _get_nc():
    global _NC_CACHE
    if _NC_CACHE is None:
        _NC_CACHE = _build()
    return _NC_CACHE


def _host_prep(inputs):
    """Build per-core in_maps from full inputs."""
    h = np.ascontiguousarray(np.asarray(inputs["h"], dtype=np.float32))
    ei = np.asarray(inputs["edge_index"]).astype(np.int64)
    src, dst = ei[0], ei[1]

    deg = np.bincount(dst, minlength=N).astype(np.float32) + 1.0
    dinv = 1.0 / np.sqrt(deg)
    coef = (dinv[src] * dinv[dst]).astype(np.float32)
    A = np.zeros((N, N), np.float32)
    np.add.at(A, (dst, src), coef)
    idx = np.arange(N)
    A[idx, idx] += dinv * dinv

    f32c = lambda k: np.asarray(inputs[k], dtype=np.float32)
    bfc = lambda x: np.ascontiguousarray(np.asarray(x).astype(BF))

    w = {k: f32c(k) for k in ("W_gcn", "Wq", "Wk", "Wv", "Wo", "RF",
                              "W1", "W2", "b_gcn", "bq", "bk", "bv", "bo",
                              "b1", "b2", "g1", "be1", "g2", "be2", "g3", "be3")}

    rfwq = w["Wq"] @ w["RF"].T          # [D, M]
    rfwk = w["Wk"] @ w["RF"].T
    wvwo = w["Wv"] @ w["Wo"]            # [D, D]
    rfbq = w["RF"] @ w["bq"]            # [M]
    rfbk = w["RF"] @ w["bk"]
    bvor = w["bv"] @ w["Wo"] + w["bo"]  # [D]

    hm8 = (h * 0.25).astype(F8).reshape(NCH, 128, D).transpose(1, 0, 2)
    common = {
        "hm": np.ascontiguousarray(hm8),
        "wgcn": bfc(w["W_gcn"]),
        "rfwq": bfc(rfwq), "rfwk": bfc(rfwk), "wvwo": bfc(wvwo),
        "w1": bfc(w["W1"]), "w2": bfc(w["W2"]),
        "rfbq": np.ascontiguousarray(rfbq.reshape(M, 1)),
        "rfbk": np.ascontiguousarray(rfbk.reshape(M, 1)),
        "bvor": bfc(bvor.reshape(1, D)),
        "b1r": bfc(w["b1"].reshape(1, F)),
        "b2r": bfc(w["b2"].reshape(1, D)),
        "g1": bfc(w["g1"].reshape(1, D)),
        "be1": bfc(w["be1"].reshape(1, D)),
        "g2": bfc(w["g2"].reshape(1, D)),
        "be2": bfc(w["be2"].reshape(1, D)),
        "g3": bfc(w["g3"].reshape(1, D)),
        "be3": bfc(w["be3"].reshape(1, D)),
    }

    in_maps = []
    for c in range(NCORES):
        r0 = c * R
        # at[rb, p, k, f] = 4*A[r0 + rb*128 + f, k*128 + p]
        a_loc = (4.0 * A[r0:r0 + R]).reshape(RB, 128, NCH, 128)
        at = np.ascontiguousarray(a_loc.transpose(0, 3, 2, 1).astype(F8))
        m = dict(common)
        m["at"] = at
        m["hres"] = np.ascontiguousarray(h[r0:r0 + R] + w["b_gcn"])
        in_maps.append(m)
    return in_maps


def kernel(**inputs):
    nc = _get_nc()
    in_maps = _host_prep(inputs)
    res = run_bass_kernel_spmd(nc, in_maps, core_ids=list(range(NCORES)))
    out = np.concatenate([np.asarray(r["out"]) for r in res.results], axis=0)
    return out.astype(np.float32)


# revision 70
# speedup vs baseline: 1.0084x; 1.0084x over previous
"""GPS layer (GCN + dense Performer attention + FFN) on 8 Trainium2 cores.

v4 strategy (per core, rows R=1024 of N=8192 nodes):
  - GCN segment-sum as dense matmul, reassociated: hl = (A @ h) @ W_gcn.
    A (x4) ships fp8-e4m3; h (x0.25) ships as fp8 value + fp8 residual, and
    the A@h contraction runs two DoubleRow chains (value + residual) so the
    fp8 quantization error of h cancels to ~0.1%.
  - Performer features qf^T/kf^T produced directly from h1^T with
    host-folded projections (Wq@RF^T, Wk@RF^T) in bf16; score matmuls bf16.
  - softmax without row-max: exp(raw/16 - 4.5) fits fp8-e5m2; the slab is
    e5m2 (1 byte -> the full [64 chunk, 1024 row] slab fits SBUF, letting
    exp start as soon as the first half of kf arrives).
  - V is host-folded through Wo (+bo): v' = h1 @ (Wv@Wo) + (bv@Wo + bo);
    softmax rows sum to 1 so +bo rides inside v'.  v' ships e4m3 PLUS its
    e4m3 residual; the P@V numerator is one 64-instruction DoubleRow
    accumulation (value part then residual part) into a single PSUM bank.
  - kf^T (bf16) and v' (2x e4m3) all-gather in two halves, fired as soon
    as their half of h1 exists.
  - ACT keeps one table per epoch: LN1 uses Sqrt (all before the first
    exp); LN2/LN3 compute rsqrt on DVE (Newton + bit-trick seed) so the
    exp table never reloads.
  - Input streaming is spread over three DMA rings: hm + at[4..7] on
    scalar, hr + weights on sync, at[0..3] on gpsimd (before any
    collective trigger blocks that ring).
  - The attention/FFN tail is software-pipelined: numerator chain rb+1
    issues between the drain of rb and the FFN tail of rb, keeping PE busy
    during the DVE layer-norm work.
"""

import os
import sys

sys.path.insert(0, "/opt/trn_rl_repo")
os.environ.setdefault("MYCRO_LOCAL_CACHE", "1")

import numpy as np
import ml_dtypes

import concourse.bass as bass
import concourse.tile as tile
from concourse import bacc, mybir
from concourse.bass_utils import run_bass_kernel_spmd
from concourse.masks import make_identity

f32 = mybir.dt.float32
bf16 = mybir.dt.bfloat16
f8e4 = mybir.dt.float8e4
f8e5 = mybir.dt.float8e5
DR = mybir.MatmulPerfMode.DoubleRow
BF = ml_dtypes.bfloat16
F8 = ml_dtypes.float8_e4m3

N, D, F, M = 8192, 256, 512, 256
NCORES = 8
R = N // NCORES          # rows per core (1024)
RB = R // 128            # row blocks per core (8)
KC = D // 128            # feature chunks (2)
NCH = N // 128           # node chunks (64)
FC = F // 128            # ffn chunks (4)
VA = 260                 # v free dim: 256 features + ones col + pad
VA2 = 2 * VA             # v' + its e4m3 residual, packed side by side
EPS = 1e-5
RH = R // 2              # rows per collective half (512)
SHIFT = -4.5             # exp shift: exp(raw/16 - 4.5) in [6e-7, 4.1e4]


def _build():
    nc = bacc.Bacc("TRN2", target_bir_lowering=False, debug=False,
                   num_devices=NCORES)

    def inp(name, shape, dt):
        return nc.dram_tensor(name, shape, dt, kind="ExternalInput")

    at_h = inp("at", [RB, 128, NCH, 128], f8e4)   # at[rb,p,k,f] = 4*A[r0+rb*128+f, k*128+p]
    hm_h = inp("hm", [128, NCH, D], f8e4)         # h * 0.25, pre-transposed p,k,d
    hres_h = inp("hres", [R, D], f32)             # h rows + b_gcn
    wgcn_h = inp("wgcn", [D, D], bf16)
    rfwq_h = inp("rfwq", [D, M], bf16)            # Wq @ RF^T
    rfwk_h = inp("rfwk", [D, M], bf16)            # Wk @ RF^T
    wvwo_h = inp("wvwo", [D, D], bf16)            # Wv @ Wo
    w1_h = inp("w1", [D, F], bf16)
    w2_h = inp("w2", [F, D], bf16)
    rfbq_h = inp("rfbq", [M, 1], f32)             # RF @ bq
    rfbk_h = inp("rfbk", [M, 1], f32)             # RF @ bk
    bvor_h = inp("bvor", [1, D], bf16)            # bv @ Wo + bo
    b1r_h = inp("b1r", [1, F], bf16)
    b2r_h = inp("b2r", [1, D], bf16)
    gb_h = {}
    for nm in ("g1", "be1", "g2", "be2", "g3", "be3"):
        gb_h[nm] = inp(nm, [1, D], bf16)

    out_h = nc.dram_tensor("out", [R, D], f32, kind="ExternalOutput")

    with tile.TileContext(nc) as tc:
        _body(tc, at_h, hm_h, hres_h, wgcn_h, rfwq_h, rfwk_h, wvwo_h,
              w1_h, w2_h, rfbq_h, rfbk_h, bvor_h, b1r_h, b2r_h, gb_h, out_h)
    nc.compile()
    return nc


def _nr_rsqrt(nc, pool, v_ap, out_ap, magic_i, W=1):
    """out = 1/sqrt(v) for [128,W] f32 via bit-trick seed + 2 Newton steps.
    All on DVE (no ACT table use). W>1 batches independent values."""
    y = pool.tile([128, 4 * W], f32, tag=f"nr{W}", name=f"nr{W}")
    yi = y.bitcast(mybir.dt.int32)
    vi = v_ap.bitcast(mybir.dt.int32)
    y0, hv, t = yi[:, 0:W], y[:, W:2 * W], y[:, 2 * W:3 * W]
    nc.vector.tensor_scalar(out=y0, in0=vi, scalar1=1, scalar2=None,
                            op0=mybir.AluOpType.arith_shift_right)
    nc.vector.scalar_tensor_tensor(out=y0, in0=magic_i, scalar=0,
                                   in1=y0,
                                   op0=mybir.AluOpType.bypass,
                                   op1=mybir.AluOpType.subtract)
    nc.vector.tensor_scalar(out=hv, in0=v_ap, scalar1=0.5, scalar2=None,
                            op0=mybir.AluOpType.mult)
    y0f = y[:, 0:W]
    for _ in range(2):
        nc.vector.tensor_mul(t, y0f, y0f)
        nc.vector.tensor_mul(t, t, hv)
        nc.vector.tensor_scalar(out=t, in0=t,
                                scalar1=-1.0, scalar2=1.5,
                                op0=mybir.AluOpType.mult,
                                op1=mybir.AluOpType.add)
        nc.vector.tensor_mul(y0f, y0f, t)
    nc.vector.tensor_copy(out_ap, y0f)


def _body(tc, at_h, hm_h, hres_h, wgcn_h, rfwq_h, rfwk_h, wvwo_h,
          w1_h, w2_h, rfbq_h, rfbk_h, bvor_h, b1r_h, b2r_h, gb_h, out_h):
    from contextlib import ExitStack
    nc = tc.nc
    Exp = mybir.ActivationFunctionType.Exp
    Sqrt = mybir.ActivationFunctionType.Sqrt
    Copy = mybir.ActivationFunctionType.Copy

    with ExitStack() as octx:
        const = octx.enter_context(tc.tile_pool(name="const", bufs=1))
        persist = octx.enter_context(tc.tile_pool(name="persist", bufs=1))
        dram = octx.enter_context(tc.tile_pool(name="dram", bufs=1, space="DRAM"))
        sc = octx.enter_context(tc.tile_pool(name="sc", bufs=2))
        big_ps = octx.enter_context(tc.tile_pool(name="big_ps", bufs=2, space="PSUM"))
        acc_ps = octx.enter_context(tc.tile_pool(name="acc_ps", bufs=1, space="PSUM"))
        tp_ps = octx.enter_context(tc.tile_pool(name="tp_ps", bufs=1, space="PSUM"))
        num_ps = octx.enter_context(tc.tile_pool(name="num_ps", bufs=1, space="PSUM"))

        # ---- const tiles (allocation only; DMA issue order is controlled
        #      below so the sync ring streams hr before the fat weights) ----
        def wtile(chunks, width, name, dt=bf16):
            return const.tile([128, chunks, width], dt, tag=name, name=name)

        wgcn_sb = wtile(KC, D, "wgcn")
        rfwq_sb = wtile(KC, M, "rfwq")
        rfwk_sb = wtile(KC, M, "rfwk")
        wvwo_sb = wtile(KC, D, "wvwo")
        w1_sb = wtile(KC, F, "w1")
        w2_sb = wtile(FC, D, "w2")

        def load_w(t, h):
            nc.sync.dma_start(t[:], h[:, :].rearrange("(c p) w -> p c w", p=128))

        rfbq_sb = const.tile([128, KC], f32, tag="rfbq")
        rfbk_sb = const.tile([128, KC], f32, tag="rfbk")
        bvor_sb = const.tile([1, D], bf16, tag="bvor")
        b1r_sb = const.tile([1, F], bf16, tag="b1r")
        b2r_sb = const.tile([1, D], bf16, tag="b2r")
        gb_sb = {nm: const.tile([128, D], bf16, tag=nm, name=nm)
                 for nm in gb_h}

        ones_k1 = const.tile([1, 128], bf16, tag="ones")
        nc.vector.memset(ones_k1[:], 1.0)
        ident_bf = const.tile([128, 128], bf16, tag="ident")
        make_identity(nc, ident_bf[:])
        eps_t = const.tile([128, 1], f32, tag="eps")
        nc.vector.memset(eps_t[:], EPS)
        shift_t = const.tile([128, 1], f32, tag="shift")
        nc.vector.memset(shift_t[:], SHIFT)
        magic_t = const.tile([128, 2], mybir.dt.int32, tag="magic")
        nc.vector.memset(magic_t[:], 0x5F3759DF)

        # ---- persistent activations ----
        # kf/qf ship as fp8 value + fp8 residual so the score matmuls run
        # three DoubleRow chains (v@v, v@r, r@v) — faster AND more accurate
        # than a single bf16 pair.
        k8_sb = persist.tile([128, KC, N], f8e4, tag="k8")
        kr_sb = persist.tile([128, KC, N], f8e4, tag="kr")
        h1_sb = persist.tile([128, RB, D], f32, tag="h1")
        h1t_sb = persist.tile([128, KC, R], bf16, tag="h1t")
        q8_sb = persist.tile([128, KC, R], f8e4, tag="q8")
        qr_sb = persist.tile([128, KC, R], f8e4, tag="qr")
        vaug_sb = persist.tile([128, NCH, VA2], f8e4, tag="vaug")
        hres_sb = persist.tile([128, RB, D], f32, tag="hres")

        # ---- collective DRAM buffers (two halves) ----
        # kft carries [k8 | kr] stacked: [2, M, RH] fp8 per half
        kft_loc = [dram.tile([2 * M, RH], f8e4, tag=f"kft_loc{h}", name=f"kft_loc{h}")
                   for h in range(2)]
        vaug_loc = [dram.tile([RH, VA2], f8e4, tag=f"vaug_loc{h}", name=f"vaug_loc{h}")
                    for h in range(2)]
        kft_all = [dram.tile([NCORES * 2 * M, RH], f8e4, tag=f"kft_all{h}",
                              name=f"kft_all{h}", addr_space="Shared")
                   for h in range(2)]
        vaug_all = [dram.tile([NCORES * RH, VA2], f8e4, tag=f"vaug_all{h}",
                               name=f"vaug_all{h}", addr_space="Shared")
                    for h in range(2)]

        def fire_kft(half):
            nc.gpsimd.collective_compute(
                "AllGather", mybir.AluOpType.bypass,
                replica_groups=[list(range(NCORES))],
                ins=[kft_loc[half][:].opt()], outs=[kft_all[half][:].opt()])

        def fire_vaug(half):
            nc.gpsimd.collective_compute(
                "AllGather", mybir.AluOpType.bypass,
                replica_groups=[list(range(NCORES))],
                ins=[vaug_loc[half][:].opt()], outs=[vaug_all[half][:].opt()])

        def load_kfa(half):
            r0 = half * RH
            for piece, dst in ((0, k8_sb), (1, kr_sb)):
                for mc in range(KC):
                    nc.sync.dma_start(
                        dst[:, mc, :].rearrange(
                            "p (c rr) -> p c rr", c=NCORES)[:, :, r0:r0 + RH],
                        kft_all[half][:, :].rearrange(
                            "(c t m p) r -> t m p c r",
                            t=2, m=KC, p=128)[piece][mc])

        def load_vaug(half):
            for c in range(NCORES):
                cg0 = c * 8 + half * 4
                nc.sync.dma_start(
                    vaug_sb[:, cg0:cg0 + 4, :],
                    vaug_all[half][c * 4 * 128:(c * 4 + 4) * 128, :].rearrange(
                        "(l p) v -> p l v", p=128))

        # ---- LN helpers ----
        def _ln_tail(x_sb, out_sb, mv, g_bc, be_bc):
            t = sc.tile([128, D], f32, tag="ln_t")
            nc.vector.scalar_tensor_tensor(
                out=t[:], in0=x_sb, scalar=mv[:, 0:1], in1=g_bc,
                op0=mybir.AluOpType.subtract, op1=mybir.AluOpType.mult)
            nc.vector.scalar_tensor_tensor(
                out=out_sb, in0=t[:], scalar=mv[:, 1:2], in1=be_bc,
                op0=mybir.AluOpType.mult, op1=mybir.AluOpType.add)

        def ln_act(x_sb, out_sb, g_bc, be_bc):
            stats = sc.tile([128, 6], f32, tag="ln_stats")
            nc.vector.bn_stats(stats[:], x_sb)
            mv = sc.tile([128, 2], f32, tag="ln_mv", bufs=4)
            nc.vector.bn_aggr(mv[:], stats[:])
            nc.scalar.activation(mv[:, 1:2], mv[:, 1:2], Sqrt, bias=eps_t[:])
            nc.vector.reciprocal(mv[:, 1:2], mv[:, 1:2])
            _ln_tail(x_sb, out_sb, mv, g_bc, be_bc)

        def ln_dve(x_sb, out_sb, g_bc, be_bc):
            stats = sc.tile([128, 6], f32, tag="ln_stats")
            nc.vector.bn_stats(stats[:], x_sb)
            mv = sc.tile([128, 2], f32, tag="ln_mv", bufs=4)
            nc.vector.bn_aggr(mv[:], stats[:])
            nc.vector.tensor_scalar(out=mv[:, 1:2], in0=mv[:, 1:2],
                                    scalar1=eps_t[:, 0:1], scalar2=None,
                                    op0=mybir.AluOpType.add)
            _nr_rsqrt(nc, sc, mv[:, 1:2], mv[:, 1:2], magic_t[:, 0:1])
            _ln_tail(x_sb, out_sb, mv, g_bc, be_bc)

        def ln_pair(xa, xb, oa, ob, g_bc, be_bc, use_act):
            """Two independent LayerNorms with interleaved stages."""
            sta = sc.tile([128, 6], f32, tag="ln_stats")
            stb = sc.tile([128, 6], f32, tag="ln_stats")
            nc.vector.bn_stats(sta[:], xa)
            nc.vector.bn_stats(stb[:], xb)
            mva = sc.tile([128, 2], f32, tag="ln_mv", bufs=4, name="mva")
            mvb = sc.tile([128, 2], f32, tag="ln_mv", bufs=4, name="mvb")
            nc.vector.bn_aggr(mva[:], sta[:])
            nc.vector.bn_aggr(mvb[:], stb[:])
            if use_act:
                nc.scalar.activation(mva[:, 1:2], mva[:, 1:2], Sqrt,
                                     bias=eps_t[:])
                nc.scalar.activation(mvb[:, 1:2], mvb[:, 1:2], Sqrt,
                                     bias=eps_t[:])
                nc.vector.reciprocal(mva[:, 1:2], mva[:, 1:2])
                nc.vector.reciprocal(mvb[:, 1:2], mvb[:, 1:2])
            else:
                vv = sc.tile([128, 2], f32, tag="vv")
                nc.vector.tensor_copy(vv[:, 0:1], mva[:, 1:2])
                nc.vector.tensor_copy(vv[:, 1:2], mvb[:, 1:2])
                nc.vector.tensor_scalar(out=vv[:], in0=vv[:],
                                        scalar1=eps_t[:, 0:1], scalar2=None,
                                        op0=mybir.AluOpType.add)
                _nr_rsqrt(nc, sc, vv[:], vv[:], magic_t[:, 0:2], W=2)
                nc.vector.tensor_copy(mva[:, 1:2], vv[:, 0:1])
                nc.vector.tensor_copy(mvb[:, 1:2], vv[:, 1:2])
            ta = sc.tile([128, D], f32, tag="ln_t")
            tb = sc.tile([128, D], f32, tag="ln_t")
            nc.vector.scalar_tensor_tensor(
                out=ta[:], in0=xa, scalar=mva[:, 0:1], in1=g_bc,
                op0=mybir.AluOpType.subtract, op1=mybir.AluOpType.mult)
            nc.vector.scalar_tensor_tensor(
                out=tb[:], in0=xb, scalar=mvb[:, 0:1], in1=g_bc,
                op0=mybir.AluOpType.subtract, op1=mybir.AluOpType.mult)
            nc.vector.scalar_tensor_tensor(
                out=oa, in0=ta[:], scalar=mva[:, 1:2], in1=be_bc,
                op0=mybir.AluOpType.mult, op1=mybir.AluOpType.add)
            nc.vector.scalar_tensor_tensor(
                out=ob, in0=tb[:], scalar=mvb[:, 1:2], in1=be_bc,
                op0=mybir.AluOpType.mult, op1=mybir.AluOpType.add)

        # ---- qkv segments ----
        def kft_half(half):
            r0 = half * RH
            for mc in range(KC):
                mm = acc_ps.tile([128, F], f32, tag="acc")
                for j in range(KC):
                    nc.tensor.matmul(mm[:, 0:RH],
                                     rfwk_sb[:, j, mc * 128:(mc + 1) * 128],
                                     h1t_sb[:, j, r0:r0 + RH],
                                     start=(j == 0), stop=(j == KC - 1))
                k8sl = sc.tile([128, RH], f8e4, tag="k8sl")
                nc.vector.tensor_scalar(
                    out=k8sl[:], in0=mm[:, 0:RH],
                    scalar1=rfbk_sb[:, mc:mc + 1], scalar2=None,
                    op0=mybir.AluOpType.add)
                krsl = sc.tile([128, RH], f8e4, tag="krsl")
                nc.vector.scalar_tensor_tensor(
                    out=krsl[:], in0=mm[:, 0:RH],
                    scalar=rfbk_sb[:, mc:mc + 1], in1=k8sl[:],
                    op0=mybir.AluOpType.add, op1=mybir.AluOpType.subtract)
                nc.sync.dma_start(
                    kft_loc[half][mc * 128:(mc + 1) * 128, :], k8sl[:])
                nc.sync.dma_start(
                    kft_loc[half][M + mc * 128:M + (mc + 1) * 128, :], krsl[:])
            fire_kft(half)

        def qft_half(half):
            r0 = half * RH
            for mc in range(KC):
                mm = acc_ps.tile([128, F], f32, tag="acc")
                for j in range(KC):
                    nc.tensor.matmul(mm[:, 0:RH],
                                     rfwq_sb[:, j, mc * 128:(mc + 1) * 128],
                                     h1t_sb[:, j, r0:r0 + RH],
                                     start=(j == 0), stop=(j == KC - 1))
                nc.vector.tensor_scalar(
                    out=q8_sb[:, mc, r0:r0 + RH], in0=mm[:, 0:RH],
                    scalar1=rfbq_sb[:, mc:mc + 1], scalar2=None,
                    op0=mybir.AluOpType.add)
                nc.vector.scalar_tensor_tensor(
                    out=qr_sb[:, mc, r0:r0 + RH], in0=mm[:, 0:RH],
                    scalar=rfbq_sb[:, mc:mc + 1], in1=q8_sb[:, mc, r0:r0 + RH],
                    op0=mybir.AluOpType.add, op1=mybir.AluOpType.subtract)

        def v_half(half):
            for b in range(half * (RB // 2), (half + 1) * (RB // 2)):
                vps = acc_ps.tile([128, F], f32, tag="acc")
                for j in range(KC):
                    nc.tensor.matmul(vps[:, 0:D],
                                     h1t_sb[:, j, b * 128:(b + 1) * 128],
                                     wvwo_sb[:, j, :],
                                     start=(j == 0), stop=False)
                nc.tensor.matmul(vps[:, 0:D], ones_k1[:], bvor_sb[:],
                                 start=False, stop=True)
                # v' in e4m3 plus its quantization residual, packed [v' | r];
                # the ones column lives in v' only.
                vrow = sc.tile([128, VA2], f8e4, tag="vrow")
                nc.vector.tensor_copy(vrow[:, 0:D], vps[:, 0:D])
                nc.vector.memset(vrow[:, D:VA], 0.0)
                nc.vector.memset(vrow[:, D:D + 1], 1.0)
                nc.vector.tensor_sub(vrow[:, VA:VA + D], vps[:, 0:D],
                                     vrow[:, 0:D])
                nc.vector.memset(vrow[:, VA + D:VA2], 0.0)
                lb = b - half * (RB // 2)
                # scalar ring: the sync ring is busy with kfa loads here
                nc.scalar.dma_start(
                    vaug_loc[half][lb * 128:(lb + 1) * 128, :], vrow[:])
            fire_vaug(half)

        # ---- scores: one (rc, half) quarter = 32 chunks, 3x fp8 DR ----
        def scores_quarter(slab_sb, rc, half):
            r0 = rc * RH
            cgs = [c * 8 + half * 4 + l for c in range(NCORES) for l in range(4)]
            for i in range(0, len(cgs), 2):
                ps = big_ps.tile([128, 2, RH], f32, tag="big")
                for t in range(2):
                    cg = cgs[i + t]
                    csl = slice(cg * 128, (cg + 1) * 128)
                    rsl = slice(r0, r0 + RH)
                    nc.tensor.matmul(ps[:, t, :], k8_sb[:, :, csl],
                                     q8_sb[:, :, rsl],
                                     start=True, stop=False, perf_mode=DR)
                    nc.tensor.matmul(ps[:, t, :], k8_sb[:, :, csl],
                                     qr_sb[:, :, rsl],
                                     start=False, stop=False, perf_mode=DR)
                    nc.tensor.matmul(ps[:, t, :], kr_sb[:, :, csl],
                                     q8_sb[:, :, rsl],
                                     start=False, stop=True, perf_mode=DR)
                cg0 = cgs[i]
                nc.scalar.activation(
                    slab_sb[:, cg0:cg0 + 2, r0:r0 + RH], ps[:], Exp,
                    bias=shift_t[:], scale=1.0 / 16.0)

        # ---- attention tail, software-pipelined over row blocks ----
        def num_chain(slab_sb, rb):
            # two interleaved DR accumulation groups (value -> bank 0,
            # residual -> bank 1); consecutive pairs share the slab lhsT
            nps = num_ps.tile([128, 2, RH], f32, tag="num", name=f"nps{rb}")
            for cg in range(0, NCH, 2):
                lhs = slab_sb[:, cg:cg + 2, rb * 128:(rb + 1) * 128]
                nc.tensor.matmul(
                    nps[:, 0, 0:VA], lhs, vaug_sb[:, cg:cg + 2, 0:VA],
                    start=(cg == 0), stop=(cg == NCH - 2), perf_mode=DR)
                nc.tensor.matmul(
                    nps[:, 1, 0:VA], lhs, vaug_sb[:, cg:cg + 2, VA:VA2],
                    start=(cg == 0), stop=(cg == NCH - 2), perf_mode=DR)
            return nps

        def drain(nps, rb):
            rec = sc.tile([128, 1], f32, tag="rec")
            nc.vector.reciprocal(rec[:], nps[:, 0, D:D + 1])
            t = sc.tile([128, D], f32, tag="numsum")
            nc.vector.scalar_tensor_tensor(
                out=t[:], in0=nps[:, 0, 0:D], scalar=rec[:, 0:1],
                in1=h1_sb[:, rb, :],
                op0=mybir.AluOpType.mult, op1=mybir.AluOpType.add)
            x2 = sc.tile([128, D], f32, tag="x2", name=f"x2_{rb}", bufs=4)
            nc.vector.scalar_tensor_tensor(
                out=x2[:], in0=nps[:, 1, 0:D], scalar=rec[:, 0:1],
                in1=t[:],
                op0=mybir.AluOpType.mult, op1=mybir.AluOpType.add)
            return x2

        def ffn_tail2(x2a, x2b, rba, rbb, use_act):
            # two row blocks with interleaved stages: while one side's
            # serial LN/copy chain waits, the other's ops fill the queues
            Relu = mybir.ActivationFunctionType.Relu

            def cp(dst, src):
                if use_act:
                    nc.scalar.activation(dst, src, Copy)
                else:
                    nc.vector.tensor_copy(dst, src)

            h2a = sc.tile([128, D], f32, tag="h2", name="h2a")
            h2b = sc.tile([128, D], f32, tag="h2", name="h2b")
            ln_pair(x2a[:], x2b[:], h2a[:], h2b[:],
                    gb_sb["g2"][:], gb_sb["be2"][:], use_act)
            h2bfa = sc.tile([128, D], bf16, tag="h2bf", name="h2bfa")
            h2bfb = sc.tile([128, D], bf16, tag="h2bf", name="h2bfb")
            cp(h2bfa[:], h2a[:])
            cp(h2bfb[:], h2b[:])
            h2Ta = sc.tile([128, KC, 128], bf16, tag="h2T", name="h2Ta")
            h2Tb = sc.tile([128, KC, 128], bf16, tag="h2T", name="h2Tb")
            for src, dst in ((h2bfa, h2Ta), (h2bfb, h2Tb)):
                for j in range(KC):
                    tp = tp_ps.tile([128, 128], bf16, tag="tp")
                    nc.tensor.transpose(tp[:], src[:, j * 128:(j + 1) * 128],
                                        ident_bf[:])
                    cp(dst[:, j, :], tp[:])

            usa = sc.tile([128, F], bf16, tag="usb", name="usa")
            usb = sc.tile([128, F], bf16, tag="usb", name="usb")
            for h2T, u_sb in ((h2Ta, usa), (h2Tb, usb)):
                ups = acc_ps.tile([128, F], f32, tag="acc")
                for j in range(KC):
                    nc.tensor.matmul(ups[:], h2T[:, j, :], w1_sb[:, j, :],
                                     start=(j == 0), stop=False)
                nc.tensor.matmul(ups[:], ones_k1[:], b1r_sb[:],
                                 start=False, stop=True)
                if use_act:
                    nc.scalar.activation(u_sb[:], ups[:], Relu)
                else:
                    nc.vector.tensor_scalar_max(u_sb[:], ups[:], 0.0)

            uTa = sc.tile([128, FC, 128], bf16, tag="uT", name="uTa")
            uTb = sc.tile([128, FC, 128], bf16, tag="uT", name="uTb")
            for u_sb, uT in ((usa, uTa), (usb, uTb)):
                for jf in range(FC):
                    tp = tp_ps.tile([128, 128], bf16, tag="tp")
                    nc.tensor.transpose(
                        tp[:], u_sb[:, jf * 128:(jf + 1) * 128], ident_bf[:])
                    cp(uT[:, jf, :], tp[:])

            x3a = sc.tile([128, D], f32, tag="x3", name="x3a")
            x3b = sc.tile([128, D], f32, tag="x3", name="x3b")
            for uT, h2, x3 in ((uTa, h2a, x3a), (uTb, h2b, x3b)):
                o2 = acc_ps.tile([128, F], f32, tag="acc")
                for jf in range(FC):
                    nc.tensor.matmul(o2[:, 0:D], uT[:, jf, :], w2_sb[:, jf, :],
                                     start=(jf == 0), stop=False)
                nc.tensor.matmul(o2[:, 0:D], ones_k1[:], b2r_sb[:],
                                 start=False, stop=True)
                nc.vector.tensor_add(x3[:], o2[:, 0:D], h2[:])
            oa = sc.tile([128, D], f32, tag="osb", name="oa")
            ob = sc.tile([128, D], f32, tag="osb", name="ob")
            ln_pair(x3a[:], x3b[:], oa[:], ob[:],
                    gb_sb["g3"][:], gb_sb["be3"][:], use_act)
            nc.sync.dma_start(out_h[rba * 128:(rba + 1) * 128, :], oa[:])
            nc.sync.dma_start(out_h[rbb * 128:(rbb + 1) * 128, :], ob[:])

        # ================= main schedule =================
        # Phase-1 pool: hm/hr/at die after the GCN; the slab pool reuses
        # their SBUF afterwards (LIFO stack allocation).
        with ExitStack() as p1ctx:
            p1 = p1ctx.enter_context(tc.tile_pool(name="p1", bufs=1))
            hm_sb = p1.tile([128, NCH, D], f8e4, tag="hm")

            # at streams as 16 half-tiles (512KB each) so chains never
            # starve long enough to drop the PE p-state: gpsimd carries
            # rb0-3, scalar rb4-7, sync stays free for hm + weights.
            # bufs=14: only the last two triggers ever wait on a buffer.
            at_ts = [p1.tile([128, NCH // 2, 128], f8e4, tag="at", bufs=14,
                             name=f"at{i}")
                     for i in range(2 * RB)]
            for rb in range(4):
                for h in range(2):
                    nc.gpsimd.dma_start(at_ts[rb * 2 + h][:],
                                        at_h[rb][:, h * 32:(h + 1) * 32, :])
            for rb in range(4, RB):
                for h in range(2):
                    nc.scalar.dma_start(at_ts[rb * 2 + h][:],
                                        at_h[rb][:, h * 32:(h + 1) * 32, :])
            # sync ring: hm first (rb0 gate), tiny consts, then weights
            for q in range(4):
                nc.sync.dma_start(hm_sb[:, q * 16:(q + 1) * 16, :],
                                  hm_h[:, q * 16:(q + 1) * 16, :])
            for j in range(KC):
                nc.sync.dma_start(rfbq_sb[:, j:j + 1],
                                  rfbq_h[j * 128:(j + 1) * 128, :])
                nc.sync.dma_start(rfbk_sb[:, j:j + 1],
                                  rfbk_h[j * 128:(j + 1) * 128, :])
            nc.sync.dma_start(bvor_sb[:], bvor_h[:, :])
            nc.sync.dma_start(b1r_sb[:], b1r_h[:, :])
            nc.sync.dma_start(b2r_sb[:], b2r_h[:, :])
            for nm, h in gb_h.items():
                bcast = bass.AP(tensor=h.ap().tensor, offset=h.ap().offset,
                                ap=[[0, 128]] + list(h.ap().ap[1:]))
                nc.sync.dma_start(gb_sb[nm][:], bcast)
            load_w(wgcn_sb, wgcn_h)
            nc.sync.dma_start(
                hres_sb[:], hres_h[:, :].rearrange("(rb p) d -> p rb d", p=128))
            load_w(rfwq_sb, rfwq_h)
            load_w(rfwk_sb, rfwk_h)
            load_w(wvwo_sb, wvwo_h)
            load_w(w1_sb, w1_h)
            load_w(w2_sb, w2_h)

            def gcn_tail(rb, hl_ps):
                hl = hl_ps[:, 0, 0:D]
                hl_bf = sc.tile([128, D], bf16, tag="hlbf")
                nc.vector.tensor_copy(hl_bf[:], hl)
                hlT = sc.tile([128, KC, 128], bf16, tag="hlT")
                for j in range(KC):
                    tp = tp_ps.tile([128, 128], bf16, tag="tp")
                    nc.tensor.transpose(tp[:], hl_bf[:, j * 128:(j + 1) * 128],
                                        ident_bf[:])
                    nc.vector.tensor_copy(hlT[:, j, :], tp[:])
                # W_gcn result lands in the unused second bank of hl_ps, so
                # consecutive GCN tails don't serialize on the acc pool
                hloc = hl_ps[:, 1, 0:D]
                for j in range(KC):
                    nc.tensor.matmul(hloc, hlT[:, j, :],
                                     wgcn_sb[:, j, :],
                                     start=(j == 0), stop=(j == KC - 1))
                x1 = sc.tile([128, D], f32, tag="x1")
                nc.vector.tensor_add(x1[:], hloc, hres_sb[:, rb, :])
                ln_act(x1[:], h1_sb[:, rb, :], gb_sb["g1"][:], gb_sb["be1"][:])
                h1bf = sc.tile([128, D], bf16, tag="h1bf")
                nc.vector.tensor_copy(h1bf[:], h1_sb[:, rb, :])
                for j in range(KC):
                    tp = tp_ps.tile([128, 128], bf16, tag="tp")
                    nc.tensor.transpose(tp[:], h1bf[:, j * 128:(j + 1) * 128],
                                        ident_bf[:])
                    nc.scalar.activation(
                        h1t_sb[:, j, rb * 128:(rb + 1) * 128], tp[:], Copy)

            # software-pipelined GCN: chain rb issues before the tail of
            # rb-1 so the DVE/ACT tail hides under the next PE chain
            prev_hl = None
            for rb in range(RB):
                hl_ps = big_ps.tile([128, 2, RH], f32, tag="big")
                hl = hl_ps[:, 0, 0:D]
                for h in range(2):
                    t = at_ts[rb * 2 + h]
                    for k in range(0, NCH // 2, 2):
                        kg = h * (NCH // 2) + k
                        nc.tensor.matmul(hl, t[:, k:k + 2, :],
                                         hm_sb[:, kg:kg + 2, :],
                                         start=(kg == 0),
                                         stop=(kg == NCH - 2),
                                         perf_mode=DR)
                if prev_hl is not None:
                    gcn_tail(rb - 1, prev_hl)
                prev_hl = hl_ps
                if rb == RB // 2:
                    kft_half(0)
                    qft_half(0)
            gcn_tail(RB - 1, prev_hl)
            # half-B projections inside p1 scope so the pool teardown
            # overlaps their PE work
            kft_half(1)
            qft_half(1)

        with ExitStack() as p2ctx:
            slabp = p2ctx.enter_context(tc.tile_pool(name="slabp", bufs=1))
            slab_sb = slabp.tile([128, NCH, R], f8e5, tag="slab")
            # kfa loads go on the sync ring only after BOTH kft stores have
            # been issued, so the B stores are not stuck behind the A loads
            load_kfa(0)
            load_kfa(1)
            # v' blocks first: they fill the PE window while the kf
            # collectives are still in flight, and get the vaug gathers
            # moving early
            v_half(0)
            v_half(1)
            scores_quarter(slab_sb, 0, 0)
            scores_quarter(slab_sb, 1, 0)
            load_vaug(0)
            load_vaug(1)
            scores_quarter(slab_sb, 0, 1)   # needs kfa-B
            scores_quarter(slab_sb, 1, 1)

            # software-pipelined paired tail: two chains + drains, then the
            # previous pair's FFN (interleaved a/b) under the next chains.
            # ACT offload only for late pairs (after the last exp retired).
            pend = None
            for p in range(RB // 2):
                a, b = 2 * p, 2 * p + 1
                npsA = num_chain(slab_sb, a)
                x2a = drain(npsA, a)
                npsB = num_chain(slab_sb, b)
                x2b = drain(npsB, b)
                if pend is not None:
                    ffn_tail2(*pend)
                # all pair tails execute after the last exp retires (nums
                # gate on exp-rc0B; exps end before the first tail runs),
                # so ACT offload is safe everywhere
                pend = (x2a, x2b, a, b, True)
            ffn_tail2(*pend)


_NC_CACHE = None


def _get_nc():
    global _NC_CACHE
    if _NC_CACHE is None:
        _NC_CACHE = _build()
    return _NC_CACHE


def _host_prep(inputs):
    """Build per-core in_maps from full inputs."""
    h = np.ascontiguousarray(np.asarray(inputs["h"], dtype=np.float32))
    ei = np.asarray(inputs["edge_index"]).astype(np.int64)
    src, dst = ei[0], ei[1]

    deg = np.bincount(dst, minlength=N).astype(np.float32) + 1.0
    dinv = 1.0 / np.sqrt(deg)
    coef = (dinv[src] * dinv[dst]).astype(np.float32)
    A = np.zeros((N, N), np.float32)
    np.add.at(A, (dst, src), coef)
    idx = np.arange(N)
    A[idx, idx] += dinv * dinv

    f32c = lambda k: np.asarray(inputs[k], dtype=np.float32)
    bfc = lambda x: np.ascontiguousarray(np.asarray(x).astype(BF))

    w = {k: f32c(k) for k in ("W_gcn", "Wq", "Wk", "Wv", "Wo", "RF",
                              "W1", "W2", "b_gcn", "bq", "bk", "bv", "bo",
                              "b1", "b2", "g1", "be1", "g2", "be2", "g3", "be3")}

    rfwq = w["Wq"] @ w["RF"].T          # [D, M]
    rfwk = w["Wk"] @ w["RF"].T
    wvwo = w["Wv"] @ w["Wo"]            # [D, D]
    rfbq = w["RF"] @ w["bq"]            # [M]
    rfbk = w["RF"] @ w["bk"]
    bvor = w["bv"] @ w["Wo"] + w["bo"]  # [D]

    hm8 = (h * 0.25).astype(F8).reshape(NCH, 128, D).transpose(1, 0, 2)
    common = {
        "hm": np.ascontiguousarray(hm8),
        "wgcn": bfc(w["W_gcn"]),
        "rfwq": bfc(rfwq), "rfwk": bfc(rfwk), "wvwo": bfc(wvwo),
        "w1": bfc(w["W1"]), "w2": bfc(w["W2"]),
        "rfbq": np.ascontiguousarray(rfbq.reshape(M, 1)),
        "rfbk": np.ascontiguousarray(rfbk.reshape(M, 1)),
        "bvor": bfc(bvor.reshape(1, D)),
        "b1r": bfc(w["b1"].reshape(1, F)),
        "b2r": bfc(w["b2"].reshape(1, D)),
        "g1": bfc(w["g1"].reshape(1, D)),
        "be1": bfc(w["be1"].reshape(1, D)),
        "g2": bfc(w["g2"].reshape(1, D)),
        "be2": bfc(w["be2"].reshape(1, D)),
        "g3": bfc(w["g3"].reshape(1, D)),
        "be3": bfc(w["be3"].reshape(1, D)),
    }

    in_maps = []
    for c in range(NCORES):
        r0 = c * R
        # at[rb, p, k, f] = 4*A[r0 + rb*128 + f, k*128 + p]
        a_loc = (4.0 * A[r0:r0 + R]).reshape(RB, 128, NCH, 128)
        at = np.ascontiguousarray(a_loc.transpose(0, 3, 2, 1).astype(F8))
        m = dict(common)
        m["at"] = at
        m["hres"] = np.ascontiguousarray(h[r0:r0 + R] + w["b_gcn"])
        in_maps.append(m)
    return in_maps


def kernel(**inputs):
    nc = _get_nc()
    in_maps = _host_prep(inputs)
    res = run_bass_kernel_spmd(nc, in_maps, core_ids=list(range(NCORES)))
    out = np.concatenate([np.asarray(r["out"]) for r in res.results], axis=0)
    return out.astype(np.float32)
